# revision 2
# baseline (speedup 1.0000x reference)
"""Trainium2 Bass kernel v2 for nn_AttnResModule2D (sparse_attention).

Same math as v1 (softmax weights are query-independent, attention collapses
to a causal running average), but:
  - V ships as bf16: halves HBM traffic, enables DVE fast modes, and feeds
    1-cycle/row bf16 matmuls.
  - The three per-row stats (sum V, sum V*g~, sum V^2) are split across
    DVE / ACT / GPSIMD(Pool) via a tunable assignment table.
  - Cumsum uses a running-sum tile: cum_k = triu@u_k + ones@usum_{k-1}
    (2 matmuls per block instead of k+1).
  - The cross-half carry is extracted with a selector-column matmul
    (row 127 of the last block's cumsum) instead of separate totals.

Sharding: core c handles batch b=c//2, s-half c%2 (512 positions); the only
cross-core dependency is the [1,1026] carry AllReduce between half-pairs.
"""

import types

import numpy as np

import concourse.bass as bass
import concourse.bacc as bacc
import concourse.mybir as mybir
from concourse.tile import TileContext

F32 = mybir.dt.float32
BF16 = mybir.dt.bfloat16
ALU = mybir.AluOpType
ACTF = mybir.ActivationFunctionType

L1 = 13          # layers incl current
B = 4
S = 1024
D = 1024
SH = 512         # s positions per core
NBLK = 4
P = 128
EPS = 1e-5
NCORES = 8
CW = D + 2       # carry vector [U_total | z_total | pad]

# ---- engine assignment tables (tuned against HW e2e) ----
# J1 = sum(V): layers here on DVE (ts bf16->f32+accum), rest ACT Copy-accum
J1_DVE_L = {0, 2, 4, 6, 8}
# J2 = sum(V*g~): DVE stt bf16-in f32-out + accum (all layers)
# J3 = sum(V^2): ACT Square-accum (all layers)
# u/cum copy engines per block: "act" or "dve"
UCOPY_ENG = ["act", "act", "act", "act"]
CUMCOPY_ENG = ["act", "act", "act", "act"]
HALF = 7         # epilogue group split


def _pin_act_tables(nc):
    """Pin every activation onto natural_log_exp_and_others so the kernel
    pays exactly one ACT table load."""
    from concourse.hw_specs import get_activation_tables
    import bass_rust as _bass_rust

    tabs = list(get_activation_tables(nc.m.arch).items())
    mine = {ACTF.Exp, ACTF.Ln, ACTF.Square, ACTF.Copy, ACTF.Identity}
    doctored = [
        (nm, set(fs) if nm == "natural_log_exp_and_others" else set(fs) - mine)
        for nm, fs in tabs
    ]

    def _patched(self):
        has_act = any(
            isinstance(i, mybir.InstActivation)
            for b in self.main_func.blocks
            for i in b.instructions
        )
        if has_act:
            _bass_rust.insert_act_table_loads(self, doctored)

    nc.insert_act_table_loads = types.MethodType(_patched, nc)


# consts_b (bf16) column layout
CB_GT = 0          # [0:1024)   g~ broadcast
CB_ID = 1024       # [1024:1152) identity
CB_TRIU = 1152     # [1152:1280) upper-tri ones (incl diag)
CB_ONES = 1280     # [1280:1408) all-ones
CB_ROW = 1408      # [1408:1536) row of ones (used as [1,128])
CB_SEL = 1536      # [1536:1537) delta_127 * mask_send column
CB_W = 1544

# consts_f (f32) column layout
CF_ID = 0          # [0:128)   identity
CF_TRIU = 128      # [128:256) triu
CF_ONES = 256      # [256:384) ones
CF_ROW = 384       # [384:512) ones row
CF_SEL = 512       # [512:513) delta_127 * mask_send
CF_MRECV = 513     # [513:514) recv mask (1 on sh=1 cores)
CF_W = 520


def build_nc(compile: bool = True, use_collective: bool = True,
             reps: int = 1) -> bass.Bass:
    nc = bacc.Bacc(
        "TRN2", target_bir_lowering=False, debug=False, num_devices=NCORES
    )

    v_d = nc.dram_tensor("v", [L1, NBLK, P, D], BF16, kind="ExternalInput").ap()
    cb_d = nc.dram_tensor("cb", [P, CB_W], BF16, kind="ExternalInput").ap()
    cf_d = nc.dram_tensor("cf", [P, CF_W], F32, kind="ExternalInput").ap()
    h_d = nc.dram_tensor("h", [SH, D], F32, kind="ExternalOutput").ap()

    with TileContext(nc, num_cores=NCORES) as tc:
        with (
            tc.tile_pool(name="const", bufs=1) as cpool,
            tc.tile_pool(name="vin", bufs=18) as vpool,
            tc.tile_pool(name="prodA", bufs=2) as prodA,
            tc.tile_pool(name="prodB", bufs=2) as prodB,
            tc.tile_pool(name="scrap", bufs=2) as scrap,
            tc.tile_pool(name="stats", bufs=2) as stpool,
            tc.tile_pool(name="diag", bufs=3) as dpool,
            tc.tile_pool(name="usb", bufs=10) as upool,
            tc.tile_pool(name="hsb", bufs=2) as hpool,
            tc.tile_pool(name="small", bufs=2) as smpool,
            tc.tile_pool(name="zsb", bufs=1) as zpool,
            tc.tile_pool(name="psU", bufs=2, space="PSUM") as psU,
            tc.tile_pool(name="psP", bufs=2, space="PSUM") as psP,
            tc.tile_pool(name="psS", bufs=2, space="PSUM") as psS,
            tc.tile_pool(name="dram", bufs=1, space="DRAM") as dram,
        ):
            # ---- constants ----
            cb = cpool.tile([P, CB_W], BF16)
            cf = cpool.tile([P, CF_W], F32)
            nc.sync.dma_start(cb[:], cb_d[:])
            nc.sync.dma_start(cf[:], cf_d[:])
            gtb = cb[:, CB_GT:CB_GT + D]
            ident_b = cb[:, CB_ID:CB_ID + P]
            triu_b = cb[:, CB_TRIU:CB_TRIU + P]
            ones_b = cb[:, CB_ONES:CB_ONES + P]
            onesrow_b = cb[0:1, CB_ROW:CB_ROW + P]
            sel_b = cb[:, CB_SEL:CB_SEL + 1]
            ident_f = cf[:, CF_ID:CF_ID + P]
            triu_f = cf[:, CF_TRIU:CF_TRIU + P]
            ones_f = cf[:, CF_ONES:CF_ONES + P]
            onesrow_f = cf[0:1, CF_ROW:CF_ROW + P]
            sel_f = cf[:, CF_SEL:CF_SEL + 1]

            z_sb = zpool.tile([P, NBLK], F32)
            zc_sb = zpool.tile([P, NBLK], F32)
            usum = upool.tile([P, D], BF16, tag="usum")
            u_sbs = []
            cum_sbs = []

            for _rep in range(reps):
                for k in range(NBLK):
                    vts = []
                    for l in range(L1):
                        vt = vpool.tile([P, D], BF16, tag="v")
                        nc.sync.dma_start(vt[:], v_d[l, k])
                        vts.append(vt)

                    sv = stpool.tile([P, 16], F32, tag="sv")
                    ssq = stpool.tile([P, 16], F32, tag="ssq")
                    vg = stpool.tile([P, 16], F32, tag="vg")
                    mu = stpool.tile([P, 16], F32, tag="mu")
                    mu2 = stpool.tile([P, 16], F32, tag="mu2")
                    mu2e = stpool.tile([P, 16], F32, tag="mu2e")
                    varq = stpool.tile([P, 16], F32, tag="varq")
                    lv = stpool.tile([P, 16], F32, tag="lv")
                    rs = stpool.tile([P, 16], F32, tag="rs")
                    sc2 = stpool.tile([P, 16], F32, tag="sc2")
                    e = stpool.tile([P, 16], F32, tag="e")
                    pu = psU.tile([P, D], F32, tag="pu")

                    def stats(l):
                        vt = vts[l]
                        # J1: sum(V)
                        if l in J1_DVE_L:
                            s1 = prodB.tile([P, D], F32, tag="s1f")
                            nc.vector.tensor_scalar(
                                s1[:], vt[:], 1.0, None, ALU.mult, ALU.add,
                                accum_out=sv[:, l:l + 1])
                        else:
                            s1 = scrap.tile([P, D], BF16, tag="s1")
                            nc.scalar.activation(
                                s1[:], vt[:], ACTF.Copy,
                                accum_out=sv[:, l:l + 1])
                        # J2: sum(V*g~) on DVE, f32 scratch out
                        s2 = prodA.tile([P, D], F32, tag="s2f")
                        nc.vector.scalar_tensor_tensor(
                            s2[:], vt[:], 1.0, gtb,
                            ALU.mult, ALU.mult,
                            accum_out=vg[:, l:l + 1])
                        # J3: sum(V^2) on ACT
                        s3 = scrap.tile([P, D], BF16, tag="s3")
                        nc.scalar.activation(
                            s3[:], vt[:], ACTF.Square,
                            accum_out=ssq[:, l:l + 1])

                    def epilogue(lo, hi):
                        c = slice(lo, hi)
                        nc.vector.tensor_scalar(
                            mu[:, c], sv[:, c], 1.0 / D, None, ALU.mult)
                        nc.vector.tensor_tensor(
                            mu2[:, c], mu[:, c], mu[:, c], ALU.mult)
                        nc.vector.tensor_scalar(
                            mu2e[:, c], mu2[:, c], EPS, None, ALU.subtract)
                        nc.vector.scalar_tensor_tensor(
                            varq[:, c], ssq[:, c], 1.0 / D, mu2e[:, c],
                            ALU.mult, ALU.subtract)
                        nc.scalar.activation(lv[:, c], varq[:, c], ACTF.Ln)
                        nc.scalar.activation(
                            rs[:, c], lv[:, c], ACTF.Exp, scale=-0.5)
                        nc.vector.tensor_tensor(
                            sc2[:, c], vg[:, c], rs[:, c], ALU.mult)
                        nc.scalar.activation(e[:, c], sc2[:, c], ACTF.Exp)

                    def umm(lo, hi):
                        for l in range(lo, hi):
                            dg = dpool.tile([P, P], BF16, tag="dg")
                            nc.vector.tensor_scalar(
                                dg[:], ident_b, e[:, l:l + 1], None, ALU.mult)
                            for n in range(2):
                                ns = slice(n * 512, (n + 1) * 512)
                                nc.tensor.matmul(
                                    pu[:, ns], dg[:], vts[l][:, ns],
                                    start=(l == 0), stop=(l == L1 - 1))

                    for l in range(HALF):
                        stats(l)
                    epilogue(0, HALF)
                    umm(0, HALF)
                    for l in range(HALF, L1):
                        stats(l)
                    epilogue(HALF, L1)
                    umm(HALF, L1)

                    nc.vector.tensor_reduce(
                        z_sb[:, k:k + 1], e[:, :L1], mybir.AxisListType.X,
                        ALU.add)
                    u_sb = upool.tile([P, D], BF16, tag="u")
                    if UCOPY_ENG[k] == "act":
                        nc.scalar.copy(u_sb[:], pu[:])
                    else:
                        nc.vector.tensor_copy(u_sb[:], pu[:])
                    u_sbs.append(u_sb)

                    # cumsum: pc = triu@u_k + ones@usum_{k-1} (per half)
                    cum_sb = upool.tile([P, D], BF16, tag="u")
                    for n in range(2):
                        ns = slice(n * 512, (n + 1) * 512)
                        pc = psP.tile([P, 512], F32, tag="pc")
                        nc.tensor.matmul(
                            pc[:], triu_b, u_sb[:, ns],
                            start=True, stop=(k == 0))
                        if k > 0:
                            nc.tensor.matmul(
                                pc[:], ones_b, usum[:, ns],
                                start=False, stop=True)
                        if CUMCOPY_ENG[k] == "act":
                            nc.scalar.copy(cum_sb[:, ns], pc[:])
                        else:
                            nc.vector.tensor_copy(cum_sb[:, ns], pc[:])
                    cum_sbs.append(cum_sb)

                    # usum += u_k  (not needed after last block)
                    if k == 0:
                        nc.vector.tensor_copy(usum[:], u_sb[:])
                    elif k < NBLK - 1:
                        nc.vector.tensor_tensor(
                            usum[:], usum[:], u_sb[:], ALU.add)

                    # z cumsum: pz = triu@z_k + ones@z_j (j<k)
                    pz = psS.tile([P, 1], F32, tag="pz")
                    nc.tensor.matmul(
                        pz[:], triu_f, z_sb[:, k:k + 1],
                        start=True, stop=(k == 0))
                    for j in range(k):
                        nc.tensor.matmul(
                            pz[:], ones_f, z_sb[:, j:j + 1],
                            start=False, stop=(j == k - 1))
                    nc.vector.tensor_copy(zc_sb[:, k:k + 1], pz[:])

                # ---- carry: row 127 of last block's cumsum (masked) ----
                carry_tx = smpool.tile([1, CW], F32, tag="ctx")
                nc.vector.memset(carry_tx[:, D:], 0.0)
                for n in range(2):
                    ns = slice(n * 512, (n + 1) * 512)
                    pcy = psP.tile([P, 512], F32, tag="pc")
                    nc.tensor.matmul(
                        pcy[0:1, :], sel_b, cum_sbs[-1][:, ns],
                        start=True, stop=True)
                    nc.vector.tensor_copy(carry_tx[:, ns], pcy[0:1, :])
                pcz = psS.tile([P, 1], F32, tag="pz")
                nc.tensor.matmul(
                    pcz[0:1, :], sel_f, zc_sb[:, NBLK - 1:NBLK],
                    start=True, stop=True)
                nc.vector.tensor_copy(carry_tx[:, D:D + 1], pcz[0:1, :])

                carry_rx = smpool.tile([1, CW], F32, tag="crx")
                if use_collective:
                    cin = dram.tile([1, CW], F32)
                    cout = dram.tile([1, CW], F32)
                    nc.gpsimd.dma_start(cin[:], carry_tx[:])
                    nc.gpsimd.collective_compute(
                        "AllReduce",
                        ALU.add,
                        replica_groups=[[0, 1], [2, 3], [4, 5], [6, 7]],
                        ins=[cin[:].opt()],
                        outs=[cout[:].opt()],
                    )
                    nc.gpsimd.dma_start(carry_rx[:], cout[:])
                else:
                    nc.vector.tensor_scalar(
                        carry_rx[:], carry_tx[:], 1.0, None, ALU.mult)
                # mask the received carry (only sh=1 cores add it)
                carry_b = smpool.tile([1, D], BF16, tag="cb")
                nc.vector.tensor_scalar(
                    carry_b[:], carry_rx[:, 0:D], cf[0:1, CF_MRECV:CF_MRECV + 1],
                    None, ALU.mult)
                carry_zf = smpool.tile([1, 1], F32, tag="czf")
                nc.vector.tensor_tensor(
                    carry_zf[:], carry_rx[:, D:D + 1],
                    cf[0:1, CF_MRECV:CF_MRECV + 1], ALU.mult)

                # keep PE warm through the collective window
                for w in range(8):
                    pw = psS.tile([P, 1], F32, tag="pz")
                    nc.tensor.matmul(
                        pw[:], ident_b, u_sbs[-1][:, 0:1],
                        start=True, stop=True)

                # ---- post-collective: add carry, normalize, store ----
                for k in range(NBLK):
                    pz2 = psS.tile([P, 1], F32, tag="pz")
                    nc.tensor.matmul(
                        pz2[:], onesrow_f, carry_zf[:],
                        start=True, stop=False)
                    nc.tensor.matmul(
                        pz2[:], ident_f, zc_sb[:, k:k + 1],
                        start=False, stop=True)
                    rz = smpool.tile([P, 1], F32, tag="rz")
                    nc.vector.reciprocal(rz[:], pz2[:])
                    h_sb = hpool.tile([P, D], F32, tag="h")
                    for n in range(2):
                        ns = slice(n * 512, (n + 1) * 512)
                        pc2 = psP.tile([P, 512], F32, tag="pc")
                        nc.tensor.matmul(
                            pc2[:], onesrow_b, carry_b[:, ns],
                            start=True, stop=False)
                        nc.tensor.matmul(
                            pc2[:], ident_b, cum_sbs[k][:, ns],
                            start=False, stop=True)
                        nc.scalar.activation(
                            h_sb[:, ns], pc2[:], ACTF.Copy, scale=rz[:])
                    nc.sync.dma_start(h_d[k * P:(k + 1) * P, :], h_sb[:])
                u_sbs = []
                cum_sbs = []

    _pin_act_tables(nc)
    if compile:
        nc.compile()
    return nc


def _host_consts(sh):
    import ml_dtypes

    cb = np.zeros((P, CB_W), dtype=np.float32)
    cb[:, CB_ID:CB_ID + P] = np.eye(P)
    cb[:, CB_TRIU:CB_TRIU + P] = np.triu(np.ones((P, P)))
    cb[:, CB_ONES:CB_ONES + P] = 1.0
    cb[0, CB_ROW:CB_ROW + P] = 1.0
    if sh == 0:
        cb[127, CB_SEL] = 1.0
    cf = np.zeros((P, CF_W), dtype=np.float32)
    cf[:, CF_ID:CF_ID + P] = np.eye(P)
    cf[:, CF_TRIU:CF_TRIU + P] = np.triu(np.ones((P, P)))
    cf[:, CF_ONES:CF_ONES + P] = 1.0
    cf[0, CF_ROW:CF_ROW + P] = 1.0
    if sh == 0:
        cf[127, CF_SEL] = 1.0
    if sh == 1:
        cf[0, CF_MRECV] = 1.0
    return cb.astype(ml_dtypes.bfloat16), cf


_NC_CACHE = {}


def get_nc():
    if "nc" not in _NC_CACHE:
        _NC_CACHE["nc"] = build_nc()
    return _NC_CACHE["nc"]


def make_in_maps(layer_history, current, w, gamma, beta):
    import ml_dtypes

    layer_history = np.asarray(layer_history)
    current = np.asarray(current)
    w = np.asarray(w, dtype=np.float64)
    gamma = np.asarray(gamma, dtype=np.float64)

    g = gamma * w
    gt = (g - g.sum() / D).astype(np.float32)

    in_maps = []
    for c in range(NCORES):
        b, sh = c // 2, c % 2
        s0 = sh * SH
        V = np.concatenate(
            [layer_history[:, b, s0:s0 + SH, :],
             current[None, b, s0:s0 + SH, :]],
            axis=0,
        ).reshape(L1, NBLK, P, D).astype(ml_dtypes.bfloat16)
        cb, cf = _host_consts(sh)
        cb[:, CB_GT:CB_GT + D] = gt[None, :].astype(ml_dtypes.bfloat16)
        in_maps.append({"v": np.ascontiguousarray(V), "cb": cb, "cf": cf})
    return in_maps


def kernel(layer_history, current, w, gamma, beta):
    import jax
    import bench

    in_maps = make_in_maps(layer_history, current, w, gamma, beta)
    if "runner" not in _NC_CACHE:
        nc = get_nc()
        fn, dev_in, dev_zero, out_names, _ = bench.make_runner(
            nc, in_maps, NCORES)
        in_names = []
        for a in nc.m.functions[0].allocations:
            if not isinstance(a, mybir.MemoryLocationSet):
                continue
            name = a.memorylocations[0].name
            if a.kind == "ExternalInput" and (
                nc.partition_id_tensor is None
                or name != nc.partition_id_tensor.name
            ):
                in_names.append(name)
        _NC_CACHE["runner"] = (fn, dev_zero, out_names, in_names)
        out_arrs = fn(*dev_in, *dev_zero)
    else:
        fn, dev_zero, out_names, in_names = _NC_CACHE["runner"]
        from jax.sharding import Mesh, PartitionSpec
        devices = jax.devices()[:NCORES]
        mesh = Mesh(np.asarray(devices), ("core",))
        sharding = jax.sharding.NamedSharding(mesh, PartitionSpec("core"))
        concat_in = [
            np.concatenate([in_maps[c][name] for c in range(NCORES)], axis=0)
            for name in in_names
        ]
        dev_in = [jax.device_put(x, sharding) for x in concat_in]
        out_arrs = fn(*dev_in, *dev_zero)
    oh = np.asarray(out_arrs[out_names.index("h")]).reshape(NCORES, SH, D)
    h = np.empty((B, S, D), dtype=np.float32)
    for c in range(NCORES):
        b, sh = c // 2, c % 2
        h[b, sh * SH:(sh + 1) * SH, :] = oh[c]
    return h


# revision 3
# speedup vs baseline: 1.8461x; 1.8461x over previous
"""Trainium2 Bass kernel v2 for nn_AttnResModule2D (sparse_attention).

Same math as v1 (softmax weights are query-independent, attention collapses
to a causal running average), but:
  - V ships as bf16: halves HBM traffic, enables DVE fast modes, and feeds
    1-cycle/row bf16 matmuls.
  - The three per-row stats (sum V, sum V*g~, sum V^2) are split across
    DVE / ACT / GPSIMD(Pool) via a tunable assignment table.
  - Cumsum uses a running-sum tile: cum_k = triu@u_k + ones@usum_{k-1}
    (2 matmuls per block instead of k+1).
  - The cross-half carry is extracted with a selector-column matmul
    (row 127 of the last block's cumsum) instead of separate totals.

Sharding: core c handles batch b=c//2, s-half c%2 (512 positions); the only
cross-core dependency is the [1,1026] carry AllReduce between half-pairs.
"""

import types

import numpy as np

import concourse.bass as bass
import concourse.bacc as bacc
import concourse.mybir as mybir
from concourse.tile import TileContext

F32 = mybir.dt.float32
BF16 = mybir.dt.bfloat16
ALU = mybir.AluOpType
ACTF = mybir.ActivationFunctionType

L1 = 13          # layers incl current
B = 4
S = 1024
D = 1024
SH = 512         # s positions per core
NBLK = 4
P = 128
EPS = 1e-5
NCORES = 8
CW = D + 2       # carry vector [U_total | z_total | pad]

# ---- engine assignment tables (tuned against HW e2e) ----
# J1 = sum(V): layers here on DVE (ts bf16->f32+accum), rest ACT Copy-accum
J1_DVE_L = {0, 2, 4, 6, 8}
# J2 = sum(V*g~): DVE stt bf16-in f32-out + accum (all layers)
# J3 = sum(V^2): ACT Square-accum (all layers)
# u/cum copy engines per block: "act" or "dve"
UCOPY_ENG = ["act", "act", "act", "act"]
CUMCOPY_ENG = ["act", "act", "act", "act"]
HALF = 7         # epilogue group split


def _pin_act_tables(nc):
    """Pin every activation onto natural_log_exp_and_others so the kernel
    pays exactly one ACT table load."""
    from concourse.hw_specs import get_activation_tables
    import bass_rust as _bass_rust

    tabs = list(get_activation_tables(nc.m.arch).items())
    mine = {ACTF.Exp, ACTF.Ln, ACTF.Square, ACTF.Copy, ACTF.Identity}
    doctored = [
        (nm, set(fs) if nm == "natural_log_exp_and_others" else set(fs) - mine)
        for nm, fs in tabs
    ]

    def _patched(self):
        has_act = any(
            isinstance(i, mybir.InstActivation)
            for b in self.main_func.blocks
            for i in b.instructions
        )
        if has_act:
            _bass_rust.insert_act_table_loads(self, doctored)

    nc.insert_act_table_loads = types.MethodType(_patched, nc)


# consts_b (bf16) column layout
CB_GT = 0          # [0:1024)   g~ broadcast
CB_ID = 1024       # [1024:1152) identity
CB_TRIU = 1152     # [1152:1280) upper-tri ones (incl diag)
CB_ONES = 1280     # [1280:1408) all-ones
CB_ROW = 1408      # [1408:1536) row of ones (used as [1,128])
CB_SEL = 1536      # [1536:1537) delta_127 * mask_send column
CB_W = 1544

# consts_f (f32) column layout
CF_ID = 0          # [0:128)   identity
CF_TRIU = 128      # [128:256) triu
CF_ONES = 256      # [256:384) ones
CF_ROW = 384       # [384:512) ones row
CF_SEL = 512       # [512:513) delta_127 * mask_send
CF_MRECV = 513     # [513:514) recv mask (1 on sh=1 cores)
CF_W = 520


def build_nc(compile: bool = True, use_collective: bool = True,
             reps: int = 1) -> bass.Bass:
    nc = bacc.Bacc(
        "TRN2", target_bir_lowering=False, debug=False, num_devices=NCORES
    )

    v_d = nc.dram_tensor("v", [L1, NBLK, P, D], BF16, kind="ExternalInput").ap()
    cb_d = nc.dram_tensor("cb", [P, CB_W], BF16, kind="ExternalInput").ap()
    cf_d = nc.dram_tensor("cf", [P, CF_W], F32, kind="ExternalInput").ap()
    h_d = nc.dram_tensor("h", [SH, D], F32, kind="ExternalOutput").ap()

    with TileContext(nc, num_cores=NCORES) as tc:
        with (
            tc.tile_pool(name="const", bufs=1) as cpool,
            tc.tile_pool(name="vin", bufs=18) as vpool,
            tc.tile_pool(name="prodA", bufs=2) as prodA,
            tc.tile_pool(name="prodB", bufs=2) as prodB,
            tc.tile_pool(name="scrap", bufs=2) as scrap,
            tc.tile_pool(name="stats", bufs=2) as stpool,
            tc.tile_pool(name="diag", bufs=3) as dpool,
            tc.tile_pool(name="usb", bufs=10) as upool,
            tc.tile_pool(name="hsb", bufs=2) as hpool,
            tc.tile_pool(name="small", bufs=2) as smpool,
            tc.tile_pool(name="zsb", bufs=1) as zpool,
            tc.tile_pool(name="psU", bufs=2, space="PSUM") as psU,
            tc.tile_pool(name="psP", bufs=2, space="PSUM") as psP,
            tc.tile_pool(name="psS", bufs=2, space="PSUM") as psS,
            tc.tile_pool(name="dram", bufs=1, space="DRAM") as dram,
        ):
            # ---- constants ----
            cb = cpool.tile([P, CB_W], BF16)
            cf = cpool.tile([P, CF_W], F32)
            nc.sync.dma_start(cb[:], cb_d[:])
            nc.sync.dma_start(cf[:], cf_d[:])
            gtb = cb[:, CB_GT:CB_GT + D]
            ident_b = cb[:, CB_ID:CB_ID + P]
            triu_b = cb[:, CB_TRIU:CB_TRIU + P]
            ones_b = cb[:, CB_ONES:CB_ONES + P]
            onesrow_b = cb[0:1, CB_ROW:CB_ROW + P]
            sel_b = cb[:, CB_SEL:CB_SEL + 1]
            ident_f = cf[:, CF_ID:CF_ID + P]
            triu_f = cf[:, CF_TRIU:CF_TRIU + P]
            ones_f = cf[:, CF_ONES:CF_ONES + P]
            onesrow_f = cf[0:1, CF_ROW:CF_ROW + P]
            sel_f = cf[:, CF_SEL:CF_SEL + 1]

            z_sb = zpool.tile([P, NBLK], F32)
            zc_sb = zpool.tile([P, NBLK], F32)
            usum = upool.tile([P, D], BF16, tag="usum")
            u_sbs = []
            cum_sbs = []

            for _rep in range(reps):
                for k in range(NBLK):
                    vts = []
                    for l in range(L1):
                        vt = vpool.tile([P, D], BF16, tag="v")
                        nc.sync.dma_start(vt[:], v_d[l, k])
                        vts.append(vt)

                    sv = stpool.tile([P, 16], F32, tag="sv")
                    ssq = stpool.tile([P, 16], F32, tag="ssq")
                    vg = stpool.tile([P, 16], F32, tag="vg")
                    mu = stpool.tile([P, 16], F32, tag="mu")
                    mu2 = stpool.tile([P, 16], F32, tag="mu2")
                    mu2e = stpool.tile([P, 16], F32, tag="mu2e")
                    varq = stpool.tile([P, 16], F32, tag="varq")
                    lv = stpool.tile([P, 16], F32, tag="lv")
                    rs = stpool.tile([P, 16], F32, tag="rs")
                    sc2 = stpool.tile([P, 16], F32, tag="sc2")
                    e = stpool.tile([P, 16], F32, tag="e")
                    pu = psU.tile([P, D], F32, tag="pu")

                    def stats(l):
                        vt = vts[l]
                        # J1: sum(V)
                        if l in J1_DVE_L:
                            s1 = prodB.tile([P, D], F32, tag="s1f")
                            nc.vector.tensor_scalar(
                                s1[:], vt[:], 1.0, None, ALU.mult, ALU.add,
                                accum_out=sv[:, l:l + 1])
                        else:
                            s1 = scrap.tile([P, D], BF16, tag="s1")
                            nc.scalar.activation(
                                s1[:], vt[:], ACTF.Copy,
                                accum_out=sv[:, l:l + 1])
                        # J2: sum(V*g~) on DVE, f32 scratch out
                        s2 = prodA.tile([P, D], F32, tag="s2f")
                        nc.vector.scalar_tensor_tensor(
                            s2[:], vt[:], 1.0, gtb,
                            ALU.mult, ALU.mult,
                            accum_out=vg[:, l:l + 1])
                        # J3: sum(V^2) on ACT
                        s3 = scrap.tile([P, D], BF16, tag="s3")
                        nc.scalar.activation(
                            s3[:], vt[:], ACTF.Square,
                            accum_out=ssq[:, l:l + 1])

                    def epilogue(lo, hi):
                        c = slice(lo, hi)
                        nc.vector.tensor_scalar(
                            mu[:, c], sv[:, c], 1.0 / D, None, ALU.mult)
                        nc.vector.tensor_tensor(
                            mu2[:, c], mu[:, c], mu[:, c], ALU.mult)
                        nc.vector.tensor_scalar(
                            mu2e[:, c], mu2[:, c], EPS, None, ALU.subtract)
                        nc.vector.scalar_tensor_tensor(
                            varq[:, c], ssq[:, c], 1.0 / D, mu2e[:, c],
                            ALU.mult, ALU.subtract)
                        nc.scalar.activation(lv[:, c], varq[:, c], ACTF.Ln)
                        nc.scalar.activation(
                            rs[:, c], lv[:, c], ACTF.Exp, scale=-0.5)
                        nc.vector.tensor_tensor(
                            sc2[:, c], vg[:, c], rs[:, c], ALU.mult)
                        nc.scalar.activation(e[:, c], sc2[:, c], ACTF.Exp)

                    def umm(lo, hi):
                        for l in range(lo, hi):
                            dg = dpool.tile([P, P], BF16, tag="dg")
                            nc.vector.tensor_scalar(
                                dg[:], ident_b, e[:, l:l + 1], None, ALU.mult)
                            for n in range(2):
                                ns = slice(n * 512, (n + 1) * 512)
                                nc.tensor.matmul(
                                    pu[:, ns], dg[:], vts[l][:, ns],
                                    start=(l == 0), stop=(l == L1 - 1))

                    for l in range(HALF):
                        stats(l)
                    epilogue(0, HALF)
                    umm(0, HALF)
                    for l in range(HALF, L1):
                        stats(l)
                    epilogue(HALF, L1)
                    umm(HALF, L1)

                    nc.vector.tensor_reduce(
                        z_sb[:, k:k + 1], e[:, :L1], mybir.AxisListType.X,
                        ALU.add)
                    u_sb = upool.tile([P, D], BF16, tag="u")
                    if UCOPY_ENG[k] == "act":
                        nc.scalar.copy(u_sb[:], pu[:])
                    else:
                        nc.vector.tensor_copy(u_sb[:], pu[:])
                    u_sbs.append(u_sb)

                    # cumsum: pc = triu@u_k + ones@usum_{k-1} (per half)
                    cum_sb = upool.tile([P, D], BF16, tag="u")
                    for n in range(2):
                        ns = slice(n * 512, (n + 1) * 512)
                        pc = psP.tile([P, 512], F32, tag="pc")
                        nc.tensor.matmul(
                            pc[:], triu_b, u_sb[:, ns],
                            start=True, stop=(k == 0))
                        if k > 0:
                            nc.tensor.matmul(
                                pc[:], ones_b, usum[:, ns],
                                start=False, stop=True)
                        if CUMCOPY_ENG[k] == "act":
                            nc.scalar.copy(cum_sb[:, ns], pc[:])
                        else:
                            nc.vector.tensor_copy(cum_sb[:, ns], pc[:])
                    cum_sbs.append(cum_sb)

                    # usum += u_k  (not needed after last block)
                    if k == 0:
                        nc.vector.tensor_copy(usum[:], u_sb[:])
                    elif k < NBLK - 1:
                        nc.vector.tensor_tensor(
                            usum[:], usum[:], u_sb[:], ALU.add)

                    # z cumsum: pz = triu@z_k + ones@z_j (j<k)
                    pz = psS.tile([P, 1], F32, tag="pz")
                    nc.tensor.matmul(
                        pz[:], triu_f, z_sb[:, k:k + 1],
                        start=True, stop=(k == 0))
                    for j in range(k):
                        nc.tensor.matmul(
                            pz[:], ones_f, z_sb[:, j:j + 1],
                            start=False, stop=(j == k - 1))
                    nc.vector.tensor_copy(zc_sb[:, k:k + 1], pz[:])

                # ---- carry: row 127 of last block's cumsum (masked) ----
                carry_tx = smpool.tile([1, CW], F32, tag="ctx")
                nc.vector.memset(carry_tx[:, D:], 0.0)
                for n in range(2):
                    ns = slice(n * 512, (n + 1) * 512)
                    pcy = psP.tile([P, 512], F32, tag="pc")
                    nc.tensor.matmul(
                        pcy[0:1, :], sel_b, cum_sbs[-1][:, ns],
                        start=True, stop=True)
                    nc.vector.tensor_copy(carry_tx[:, ns], pcy[0:1, :])
                pcz = psS.tile([P, 1], F32, tag="pz")
                nc.tensor.matmul(
                    pcz[0:1, :], sel_f, zc_sb[:, NBLK - 1:NBLK],
                    start=True, stop=True)
                nc.vector.tensor_copy(carry_tx[:, D:D + 1], pcz[0:1, :])

                carry_rx = smpool.tile([1, CW], F32, tag="crx")
                if use_collective:
                    cin = dram.tile([1, CW], F32)
                    cout = dram.tile([1, CW], F32)
                    nc.gpsimd.dma_start(cin[:], carry_tx[:])
                    nc.gpsimd.collective_compute(
                        "AllReduce",
                        ALU.add,
                        replica_groups=[[0, 1], [2, 3], [4, 5], [6, 7]],
                        ins=[cin[:].opt()],
                        outs=[cout[:].opt()],
                    )
                    nc.gpsimd.dma_start(carry_rx[:], cout[:])
                else:
                    nc.vector.tensor_scalar(
                        carry_rx[:], carry_tx[:], 1.0, None, ALU.mult)
                # mask the received carry (only sh=1 cores add it)
                carry_b = smpool.tile([1, D], BF16, tag="cb")
                nc.vector.tensor_scalar(
                    carry_b[:], carry_rx[:, 0:D], cf[0:1, CF_MRECV:CF_MRECV + 1],
                    None, ALU.mult)
                carry_zf = smpool.tile([1, 1], F32, tag="czf")
                nc.vector.tensor_tensor(
                    carry_zf[:], carry_rx[:, D:D + 1],
                    cf[0:1, CF_MRECV:CF_MRECV + 1], ALU.mult)

                # keep PE warm through the collective window
                for w in range(8):
                    pw = psS.tile([P, 1], F32, tag="pz")
                    nc.tensor.matmul(
                        pw[:], ident_b, u_sbs[-1][:, 0:1],
                        start=True, stop=True)

                # ---- post-collective: add carry, normalize, store ----
                for k in range(NBLK):
                    pz2 = psS.tile([P, 1], F32, tag="pz")
                    nc.tensor.matmul(
                        pz2[:], onesrow_f, carry_zf[:],
                        start=True, stop=False)
                    nc.tensor.matmul(
                        pz2[:], ident_f, zc_sb[:, k:k + 1],
                        start=False, stop=True)
                    rz = smpool.tile([P, 1], F32, tag="rz")
                    nc.vector.reciprocal(rz[:], pz2[:])
                    h_sb = hpool.tile([P, D], F32, tag="h")
                    for n in range(2):
                        ns = slice(n * 512, (n + 1) * 512)
                        pc2 = psP.tile([P, 512], F32, tag="pc")
                        nc.tensor.matmul(
                            pc2[:], onesrow_b, carry_b[:, ns],
                            start=True, stop=False)
                        nc.tensor.matmul(
                            pc2[:], ident_b, cum_sbs[k][:, ns],
                            start=False, stop=True)
                        nc.scalar.activation(
                            h_sb[:, ns], pc2[:], ACTF.Copy, scale=rz[:])
                    nc.sync.dma_start(h_d[k * P:(k + 1) * P, :], h_sb[:])
                u_sbs = []
                cum_sbs = []

    _pin_act_tables(nc)
    if compile:
        nc.compile()
    return nc


def _host_consts(sh):
    import ml_dtypes

    cb = np.zeros((P, CB_W), dtype=np.float32)
    cb[:, CB_ID:CB_ID + P] = np.eye(P)
    cb[:, CB_TRIU:CB_TRIU + P] = np.triu(np.ones((P, P)))
    cb[:, CB_ONES:CB_ONES + P] = 1.0
    cb[0, CB_ROW:CB_ROW + P] = 1.0
    if sh == 0:
        cb[127, CB_SEL] = 1.0
    cf = np.zeros((P, CF_W), dtype=np.float32)
    cf[:, CF_ID:CF_ID + P] = np.eye(P)
    cf[:, CF_TRIU:CF_TRIU + P] = np.triu(np.ones((P, P)))
    cf[:, CF_ONES:CF_ONES + P] = 1.0
    cf[0, CF_ROW:CF_ROW + P] = 1.0
    if sh == 0:
        cf[127, CF_SEL] = 1.0
    if sh == 1:
        cf[0, CF_MRECV] = 1.0
    return cb.astype(ml_dtypes.bfloat16), cf


_NC_CACHE = {}


def get_nc():
    if "nc" not in _NC_CACHE:
        _NC_CACHE["nc"] = build_nc()
    return _NC_CACHE["nc"]


def make_in_maps(layer_history, current, w, gamma, beta):
    import ml_dtypes

    layer_history = np.asarray(layer_history)
    current = np.asarray(current)
    w = np.asarray(w, dtype=np.float64)
    gamma = np.asarray(gamma, dtype=np.float64)

    g = gamma * w
    gt = (g - g.sum() / D).astype(np.float32)

    in_maps = []
    for c in range(NCORES):
        b, sh = c // 2, c % 2
        s0 = sh * SH
        V = np.concatenate(
            [layer_history[:, b, s0:s0 + SH, :],
             current[None, b, s0:s0 + SH, :]],
            axis=0,
        ).reshape(L1, NBLK, P, D).astype(ml_dtypes.bfloat16)
        cb, cf = _host_consts(sh)
        cb[:, CB_GT:CB_GT + D] = gt[None, :].astype(ml_dtypes.bfloat16)
        in_maps.append({"v": np.ascontiguousarray(V), "cb": cb, "cf": cf})
    return in_maps




def _make_runner(nc, in_maps, ncores):
    """Build a jitted shard_map executor for the compiled bass module.
    Self-contained (no sibling imports): the harness calls kernel() with
    only this file present."""
    import jax
    from jax.sharding import Mesh, PartitionSpec
    from jax.experimental.shard_map import shard_map
    from concourse import bass2jax
    from concourse.bass2jax import _bass_exec_p, install_neuronx_cc_hook

    install_neuronx_cc_hook()
    partition_name = (
        nc.partition_id_tensor.name if nc.partition_id_tensor else None
    )
    in_names, out_names, out_avals, zero_outs = [], [], [], []
    for alloc in nc.m.functions[0].allocations:
        if not isinstance(alloc, mybir.MemoryLocationSet):
            continue
        name = alloc.memorylocations[0].name
        if alloc.kind == "ExternalInput":
            if name != partition_name:
                in_names.append(name)
        elif alloc.kind == "ExternalOutput":
            out_names.append(name)
            shape = tuple(alloc.tensor_shape)
            dtype = mybir.dt.np(alloc.dtype)
            out_avals.append(jax.core.ShapedArray(shape, dtype))
            zero_outs.append(np.zeros(shape, dtype))
    n_params = len(in_names)
    all_in = list(in_names) + out_names
    if partition_name is not None:
        all_in.append(partition_name)

    def _body(*args):
        operands = list(args)
        if partition_name is not None:
            operands.append(bass2jax.partition_id_tensor())
        outs = _bass_exec_p.bind(
            *operands,
            out_avals=tuple(out_avals),
            in_names=tuple(all_in),
            out_names=tuple(out_names),
            lowering_input_output_aliases=(),
            sim_require_finite=True,
            sim_require_nnan=True,
            nc=nc,
        )
        return tuple(outs)

    devices = jax.devices()[:ncores]
    mesh = Mesh(np.asarray(devices), ("core",))
    n_outs = len(out_avals)
    in_specs = (PartitionSpec("core"),) * (n_params + n_outs)
    out_specs = (PartitionSpec("core"),) * n_outs
    fn = jax.jit(shard_map(_body, mesh=mesh, in_specs=in_specs,
                           out_specs=out_specs, check_rep=False))
    sharding = jax.sharding.NamedSharding(mesh, PartitionSpec("core"))
    concat_in = [
        np.concatenate([in_maps[c][name] for c in range(ncores)], axis=0)
        for name in in_names
    ]
    dev_in = [jax.device_put(x, sharding) for x in concat_in]
    dev_zero = [
        jax.device_put(
            np.zeros((ncores * z.shape[0], *z.shape[1:]), z.dtype), sharding
        )
        for z in zero_outs
    ]
    return fn, dev_in, dev_zero, out_names, in_names

def kernel(layer_history, current, w, gamma, beta):
    import jax

    in_maps = make_in_maps(layer_history, current, w, gamma, beta)
    if "runner" not in _NC_CACHE:
        nc = get_nc()
        fn, dev_in, dev_zero, out_names, in_names = _make_runner(
            nc, in_maps, NCORES)
        _NC_CACHE["runner"] = (fn, dev_zero, out_names, in_names)
        out_arrs = fn(*dev_in, *dev_zero)
    else:
        fn, dev_zero, out_names, in_names = _NC_CACHE["runner"]
        from jax.sharding import Mesh, PartitionSpec
        devices = jax.devices()[:NCORES]
        mesh = Mesh(np.asarray(devices), ("core",))
        sharding = jax.sharding.NamedSharding(mesh, PartitionSpec("core"))
        concat_in = [
            np.concatenate([in_maps[c][name] for c in range(NCORES)], axis=0)
            for name in in_names
        ]
        dev_in = [jax.device_put(x, sharding) for x in concat_in]
        out_arrs = fn(*dev_in, *dev_zero)
    oh = np.asarray(out_arrs[out_names.index("h")]).reshape(NCORES, SH, D)
    h = np.empty((B, S, D), dtype=np.float32)
    for c in range(NCORES):
        b, sh = c // 2, c % 2
        h[b, sh * SH:(sh + 1) * SH, :] = oh[c]
    return h
